# revision 5
# baseline (speedup 1.0000x reference)
"""Trainium2 Bass kernel for the BAN (bilinear attention network) problem.

Math (per batch b, eval mode):
    hq = emb[he_ques] @ Wq + bq                  [NQ, H]
    hk = emb[he_kg]   @ Wk + bk                  [NK, H]
    logits[g,q,k] = sum_d hq[q,d] Watt[d,g] hk[k,d]   (+ batt[g], which cancels
                                                       in the joint softmax)
    att = softmax over flattened (q,k) per (b,g)
    pooled[g,d] = sum_{q,k} hq[q,d] att[g,q,k] hk[k,d]
    out = pooled.flat @ Wout + bout;  sim = out @ glove.T;  log_softmax(sim)

Distribution: pure data parallel over batch, 8 samples per core on 8 cores.
All weights replicated. No collectives.

All matmul operands are bf16 (the hardware runs 4-byte matmuls in the slow
multi-pass fp32 mode regardless of float32r; bf16 is ~3x faster and the
end-to-end rel err stays ~2e-3 vs the 2e-2 gate since PSUM accumulation is
fp32). Embedding table padded to 384 cols (ones column at 300 for the bias
trick, zeros beyond) so every E-chunk is a full 128 rows.

Layout strategy (per core, B_loc=8):
  - X tokens gathered on-device via indirect DMA: xrow [token, 384] bf16,
    PE-transposed (bf16 identity, bf16 PSUM) into X.T [E, token].
  - hqT [d, tok] and hkT [d, tok] from lhsT=W chunk, rhs=X.T (PSUM fp32,
    copied to SBUF as bf16).
  - hk [tok, d] recomputed from lhsT=X.T chunks, rhs=W (token-major).
  - logits.T [k, (g,q)] = (hkT tiles).T @ (hqT * Watt[:,g]) -> logits are
    O(+-6), so exp() without max subtraction is safe; the joint softmax
    normalization Z_g is applied to pooled.T per sample.
  - Z via gpsimd partition_all_reduce (no ones-matmul / broadcast needed).
  - u.T [d, (g,q)] = (hk tiles).T @ E.T;  v = u.T * hqT (bcast over g);
    pooled.T[d, g] = reduce_q v, scaled by 1/Z_g.
  - out [8, 300] = (pooled.T as lhsT).T @ Wout tiles;  sim [8, 4000] via
    lhsT=out.T (PE transpose), rhs=glove.T;  log-softmax on [8, 4000].
  - All Wout/glove tiles fit in SBUF in bf16; their DMAs are emitted
    up-front so they stream in while the attention loop runs and phases F/G
    run back-to-back on a warm PE.
"""

import sys

if "/opt/trn_rl_repo" not in sys.path:
    sys.path.insert(0, "/opt/trn_rl_repo")

import numpy as np

import concourse.bass as bass
import concourse.bass_isa as bass_isa
import concourse.mybir as mybir
import concourse.tile as tile
from concourse import bacc
from concourse.bass_utils import run_bass_kernel_spmd

F32 = mybir.dt.float32
BF16 = mybir.dt.bfloat16
I32 = mybir.dt.int32
AX = mybir.AxisListType
OP = mybir.AluOpType
AF = mybir.ActivationFunctionType

N_CORES = 8
VOCAB = 20000
E = 300          # word embedding size
EA = 384         # padded: col 300 = ones (bias trick), 301.. = zeros
E_CH = 3         # E-chunks of 128
H = 1024         # hidden
G = 8            # heads
N_OUT = 300
N_ANS = 4000
B, NQ, NK = 64, 32, 256
BL = B // N_CORES            # 8 samples per core
TQ = BL * NQ                 # 256 q tokens per core
TK = BL * NK                 # 2048 k tokens per core
TQ_TILES = TQ // 128         # 2
TK_TILES = TK // 128         # 16
DT = H // 128                # 8 d-tiles
N_CHUNKS = (128, 128, N_OUT - 256)
NA_CH = 8                    # sim computed in 8 chunks of 500
NA_W = N_ANS // NA_CH        # 500
NWOUT = G * DT               # 64 Wout k-tiles


def build_kernel():
    nc = bacc.Bacc("TRN2", target_bir_lowering=False, debug=False,
                   num_devices=N_CORES)

    # ---- DRAM I/O ----
    emb_d = nc.dram_tensor("emb", [VOCAB, EA], BF16, kind="ExternalInput").ap()
    idxq_d = nc.dram_tensor("idx_q", [128, TQ_TILES], I32, kind="ExternalInput").ap()
    idxk_d = nc.dram_tensor("idx_k", [128, TK_TILES], I32, kind="ExternalInput").ap()
    wq_d = nc.dram_tensor("wq", [EA, H], BF16, kind="ExternalInput").ap()
    wk_d = nc.dram_tensor("wk", [EA, H], BF16, kind="ExternalInput").ap()
    watt_d = nc.dram_tensor("watt", [128, DT, G], BF16, kind="ExternalInput").ap()
    wout_d = nc.dram_tensor("wout", [G * H, N_OUT], BF16, kind="ExternalInput").ap()
    bout_d = nc.dram_tensor("bout", [BL, N_OUT], F32, kind="ExternalInput").ap()
    # glove packed per (row-chunk, answer-chunk) into contiguous [128, 500]
    glovet_d = nc.dram_tensor("glovet", [E_CH, NA_CH, 128, NA_W], BF16,
                              kind="ExternalInput").ap()
    ident_d = nc.dram_tensor("ident", [128, 128], BF16, kind="ExternalInput").ap()
    out_d = nc.dram_tensor("out", [BL, N_ANS], F32, kind="ExternalOutput").ap()
    warm_d = nc.dram_tensor("warm", [1, 128], F32, kind="ExternalOutput").ap()

    with tile.TileContext(nc) as tc:
        import contextlib

        with contextlib.ExitStack() as ctx:
            consts = ctx.enter_context(tc.tile_pool(name="consts", bufs=1))
            wout_p = ctx.enter_context(tc.tile_pool(name="wout", bufs=NWOUT))
            glove_p = ctx.enter_context(tc.tile_pool(name="glove", bufs=NA_CH))
            xrow_p = ctx.enter_context(tc.tile_pool(name="xrow", bufs=6))
            xkt_p = ctx.enter_context(tc.tile_pool(name="xkt", bufs=2))
            hkt_p = ctx.enter_context(tc.tile_pool(name="hkt", bufs=2))
            hk_p = ctx.enter_context(tc.tile_pool(name="hk", bufs=2))
            hqw_p = ctx.enter_context(tc.tile_pool(name="hqw", bufs=2))
            et_p = ctx.enter_context(tc.tile_pool(name="et", bufs=2))
            v_p = ctx.enter_context(tc.tile_pool(name="v", bufs=2))
            zz_p = ctx.enter_context(tc.tile_pool(name="zz", bufs=2))
            mm_p = ctx.enter_context(tc.tile_pool(name="mm", bufs=2, space="PSUM"))
            tp_p = ctx.enter_context(tc.tile_pool(name="tp", bufs=2, space="PSUM"))
            lg_p = ctx.enter_context(tc.tile_pool(name="lg", bufs=2, space="PSUM"))
            up_p = ctx.enter_context(tc.tile_pool(name="up", bufs=2, space="PSUM"))

            # ---- constants into SBUF ----
            ident = consts.tile([128, 128], BF16, tag="ident")
            nc.sync.dma_start(ident[:], ident_d)
            # PE warm-up: back-to-back matmuls on the identity while the
            # initial DMAs stream in, so HAM reaches K=8/8 before real work
            wps = mm_p.tile([128, 512], F32, tag="mm")
            for i in range(48):
                nc.tensor.matmul(wps[:, :128], lhsT=ident[:], rhs=ident[:],
                                 start=True, stop=True)
            warm_sb = consts.tile([1, 128], F32, tag="warm")
            nc.vector.tensor_copy(warm_sb[:], wps[:1, :128])
            nc.sync.dma_start(warm_d, warm_sb[:])

            idxq_sb = consts.tile([128, TQ_TILES], I32, tag="idxq")
            nc.sync.dma_start(idxq_sb[:], idxq_d)
            idxk_sb = consts.tile([128, TK_TILES], I32, tag="idxk")
            nc.sync.dma_start(idxk_sb[:], idxk_d)
            wq_sb = consts.tile([128, E_CH, H], BF16, tag="wq")
            wk_sb = consts.tile([128, E_CH, H], BF16, tag="wk")
            for c in range(E_CH):
                nc.sync.dma_start(wq_sb[:, c, :], wq_d[c * 128 : (c + 1) * 128])
                nc.sync.dma_start(wk_sb[:, c, :], wk_d[c * 128 : (c + 1) * 128])
            watt_sb = consts.tile([128, DT, G], BF16, tag="watt")
            nc.sync.dma_start(watt_sb[:], watt_d)
            bout_sb = consts.tile([BL, N_OUT], F32, tag="bout")
            nc.sync.dma_start(bout_sb[:], bout_d)

            # ---- full weight streams for phases F/G (fit in SBUF in bf16;
            # they trickle in behind wq/wk on the SP queue during attention)
            wout_tiles = []
            for t in range(NWOUT):
                wtile = wout_p.tile([128, N_OUT], BF16, tag="wout")
                nc.sync.dma_start(wtile[:], wout_d[t * 128 : (t + 1) * 128, :])
                wout_tiles.append(wtile)
            glove_tiles = []
            for a in range(NA_CH):
                gt = glove_p.tile([128, E_CH, NA_W], BF16, tag="glove")
                for c in range(E_CH):
                    nc.sync.dma_start(gt[:, c, :], glovet_d[c, a])
                glove_tiles.append(gt)

            def gather_transpose(idx_sb, col, dst, dst_col):
                """Gather 128 emb rows (token tile) and PE-transpose them
                into dst[:, c, dst_col*128:...] per E-chunk c."""
                xrow = xrow_p.tile([128, EA], BF16, tag="xrow")
                nc.gpsimd.indirect_dma_start(
                    out=xrow[:],
                    out_offset=None,
                    in_=emb_d,
                    in_offset=bass.IndirectOffsetOnAxis(
                        ap=idx_sb[:, col : col + 1], axis=0
                    ),
                )
                for c in range(E_CH):
                    ps = tp_p.tile([128, 128], BF16, tag="tp")
                    nc.tensor.transpose(
                        ps[:], xrow[:, c * 128 : (c + 1) * 128], ident[:]
                    )
                    nc.any.tensor_copy(
                        out=dst[:, c, dst_col * 128 : (dst_col + 1) * 128],
                        in_=ps[:],
                    )

            # ---- phase B: gather+transpose Xq -> xqT [128, 3, TQ] ----
            xqT = consts.tile([128, E_CH, TQ], BF16, tag="xqT")
            for t in range(TQ_TILES):
                gather_transpose(idxq_sb, t, xqT, t)

            # ---- phase C: hqT [128, DT, TQ] bf16 ----
            hqT = consts.tile([128, DT, TQ], BF16, tag="hqT")
            for m in range(DT):
                ps = mm_p.tile([128, 512], F32, tag="mm")
                for c in range(E_CH):
                    nc.tensor.matmul(
                        ps[:, :TQ],
                        lhsT=wq_sb[:, c, m * 128 : (m + 1) * 128],
                        rhs=xqT[:, c, :],
                        start=(c == 0),
                        stop=(c == E_CH - 1),
                    )
                nc.vector.tensor_copy(hqT[:, m, :], ps[:, :TQ])

            poT = consts.tile([128, DT, G, BL], BF16, tag="poT")

            # ---- phase D: attention, two samples per D2 batch ----
            for p in range(BL // 2):
                # D1: gather + transpose K tokens for samples 2p, 2p+1
                xkT = xkt_p.tile([128, E_CH, 512], BF16, tag="xkT")
                for t in range(4):
                    gather_transpose(idxk_sb, p * 4 + t, xkT, t)

                # D2: hkT for the pair [128, DT, 512] bf16
                hkT = hkt_p.tile([128, DT, 512], BF16, tag="hkT")
                for m in range(DT):
                    ps = mm_p.tile([128, 512], F32, tag="mm")
                    for c in range(E_CH):
                        nc.tensor.matmul(
                            ps[:],
                            lhsT=wk_sb[:, c, m * 128 : (m + 1) * 128],
                            rhs=xkT[:, c, :],
                            start=(c == 0),
                            stop=(c == E_CH - 1),
                        )
                    nc.any.tensor_copy(out=hkT[:, m, :], in_=ps[:])

                for bi in range(2):
                    b = p * 2 + bi

                    # D3: hk_b [128, 2, H] (token-partition layout)
                    hk = hk_p.tile([128, 2, H], BF16, tag="hk")
                    for t in range(2):
                        for nchunk in range(2):
                            ps = mm_p.tile([128, 512], F32, tag="mm")
                            for c in range(E_CH):
                                nc.tensor.matmul(
                                    ps[:],
                                    lhsT=xkT[
                                        :, c,
                                        (bi * 2 + t) * 128 : (bi * 2 + t + 1) * 128,
                                    ],
                                    rhs=wk_sb[
                                        :, c, nchunk * 512 : (nchunk + 1) * 512
                                    ],
                                    start=(c == 0),
                                    stop=(c == E_CH - 1),
                                )
                            nc.any.tensor_copy(
                                out=hk[:, t, nchunk * 512 : (nchunk + 1) * 512],
                                in_=ps[:],
                            )

                    # D4: hqw [128, DT, G, NQ] = hqT(b) * watt (bcast over g)
                    hqw = hqw_p.tile([128, DT, G, NQ], BF16, tag="hqw")
                    nc.vector.tensor_tensor(
                        out=hqw[:],
                        in0=hqT[:, :, None, b * NQ : (b + 1) * NQ].to_broadcast(
                            [128, DT, G, NQ]
                        ),
                        in1=watt_sb[:, :, :, None].to_broadcast([128, DT, G, NQ]),
                        op=OP.mult,
                    )

                    # D5: logits.T [k, (g,q)] in PSUM: [128, 2, 256]
                    ps_l = lg_p.tile([128, 512], F32, tag="lg")
                    for kt in range(2):
                        for c in range(DT):
                            nc.tensor.matmul(
                                ps_l[:, kt * 256 : (kt + 1) * 256],
                                lhsT=hkT[
                                    :, c,
                                    bi * 256 + kt * 128 : bi * 256 + (kt + 1) * 128,
                                ],
                                rhs=hqw[:, c],
                                start=(c == 0),
                                stop=(c == DT - 1),
                            )

                    # D6: E = exp(logits) in bf16, per-(kt,g) row sums zz
                    et = et_p.tile([128, 2, G * NQ], BF16, tag="et")
                    zz = zz_p.tile([128, 2, G], F32, tag="zz")
                    for kt in range(2):
                        nc.scalar.activation(
                            out=et[:, kt, :],
                            in_=ps_l[:, kt * 256 : (kt + 1) * 256],
                            func=AF.Exp,
                        )
                        nc.vector.tensor_reduce(
                            out=zz[:, kt, :],
                            in_=et[:, kt].rearrange("p (g q) -> p g q", g=G),
                            axis=AX.X,
                            op=OP.add,
                        )

                    # D7: Z_g = sum over kt and partitions; zinv on all
                    # partitions via gpsimd partition all-reduce
                    zsum = zz_p.tile([128, G], F32, tag="zsum")
                    nc.vector.tensor_tensor(
                        out=zsum[:], in0=zz[:, 0, :], in1=zz[:, 1, :], op=OP.add
                    )
                    zall = zz_p.tile([128, G], F32, tag="zall")
                    nc.gpsimd.partition_all_reduce(
                        zall[:], zsum[:], channels=128,
                        reduce_op=bass_isa.ReduceOp.add,
                    )
                    zinv = zz_p.tile([128, G], F32, tag="zinv")
                    nc.vector.reciprocal(zinv[:], zall[:])

                    # D8: u.T, v, pooled partial sums; 2 d-tiles per PSUM tile
                    for mp in range(4):
                        ps_u = up_p.tile([128, 512], F32, tag="up")
                        for mi in range(2):
                            m = mp * 2 + mi
                            for kt in range(2):
                                nc.tensor.matmul(
                                    ps_u[:, mi * 256 : (mi + 1) * 256],
                                    lhsT=hk[:, kt, m * 128 : (m + 1) * 128],
                                    rhs=et[:, kt, :],
                                    start=(kt == 0),
                                    stop=(kt == 1),
                                )
                        v = v_p.tile([128, 2, G, NQ], BF16, tag="v")
                        nc.vector.tensor_tensor(
                            out=v[:],
                            in0=ps_u[:].rearrange("p (m g q) -> p m g q", m=2, g=G),
                            in1=hqT[
                                :, mp * 2 : mp * 2 + 2, None, b * NQ : (b + 1) * NQ
                            ].to_broadcast([128, 2, G, NQ]),
                            op=OP.mult,
                        )
                        vr = v_p.tile([128, 2, G], F32, tag="vr")
                        nc.vector.tensor_reduce(
                            out=vr[:], in_=v[:], axis=AX.X, op=OP.add
                        )
                        nc.vector.tensor_tensor(
                            out=poT[:, mp * 2 : mp * 2 + 2, :, b],
                            in0=vr[:],
                            in1=zinv[:, None, :].to_broadcast([128, 2, G]),
                            op=OP.mult,
                        )

            # ---- phase F: out [8, 300] = pooled_flat @ Wout + bout ----
            ps_o = mm_p.tile([128, 512], F32, tag="mm")
            for g in range(G):
                for m in range(DT):
                    t = g * DT + m
                    nc.tensor.matmul(
                        ps_o[:BL, :N_OUT],
                        lhsT=poT[:, m, g, :],
                        rhs=wout_tiles[t][:],
                        start=(t == 0),
                        stop=(t == NWOUT - 1),
                    )
            out_sb = consts.tile([BL, N_OUT], BF16, tag="out_sb")
            nc.vector.tensor_tensor(
                out=out_sb[:], in0=ps_o[:BL, :N_OUT], in1=bout_sb[:], op=OP.add
            )

            # ---- phase G: sim + log_softmax ----
            outT = consts.tile([128, E_CH, BL], BF16, tag="outT")
            for c, rows in enumerate(N_CHUNKS):
                ps = tp_p.tile([128, 128], BF16, tag="tp")
                nc.tensor.transpose(
                    ps[:rows, :BL],
                    out_sb[:, c * 128 : c * 128 + rows],
                    ident[:BL, :BL],
                )
                nc.vector.tensor_copy(outT[:rows, c, :], ps[:rows, :BL])

            sim_sb = consts.tile([BL, N_ANS], F32, tag="sim_sb")
            esc = consts.tile([BL, NA_W], F32, tag="esc")
            mx8 = consts.tile([BL, NA_CH], F32, tag="mx8")
            zs8 = consts.tile([BL, NA_CH], F32, tag="zs8")
            mx = consts.tile([BL, 1], F32, tag="mx")
            nmx = consts.tile([BL, 1], F32, tag="nmx")
            zs = consts.tile([BL, 1], F32, tag="zs")
            lnz = consts.tile([BL, 1], F32, tag="lnz")
            for a in range(NA_CH):
                ps_s = mm_p.tile([128, 512], F32, tag="mm")
                for c, rows in enumerate(N_CHUNKS):
                    nc.tensor.matmul(
                        ps_s[:BL, :NA_W],
                        lhsT=outT[:rows, c, :],
                        rhs=glove_tiles[a][:rows, c, :],
                        start=(c == 0),
                        stop=(c == 2),
                    )
                nc.vector.tensor_reduce(
                    out=mx8[:, a : a + 1], in_=ps_s[:BL, :NA_W], axis=AX.X, op=OP.max
                )
                nc.vector.tensor_copy(sim_sb[:, a * NA_W : (a + 1) * NA_W],
                                      ps_s[:BL, :NA_W])
            nc.vector.tensor_reduce(out=mx[:], in_=mx8[:], axis=AX.X, op=OP.max)
            nc.vector.tensor_scalar_mul(nmx[:], mx[:], -1.0)
            for a in range(NA_CH):
                nc.scalar.activation(
                    out=esc[:],  # scratch, discarded
                    in_=sim_sb[:, a * NA_W : (a + 1) * NA_W],
                    func=AF.Exp,
                    bias=nmx[:],
                    accum_out=zs8[:, a : a + 1],
                )
            nc.vector.tensor_reduce(out=zs[:], in_=zs8[:], axis=AX.X, op=OP.add)
            nc.scalar.activation(out=lnz[:], in_=zs[:], func=AF.Ln)
            nc.vector.tensor_scalar(
                out=sim_sb[:],
                in0=sim_sb[:],
                scalar1=mx[:],
                scalar2=lnz[:],
                op0=OP.subtract,
                op1=OP.subtract,
            )
            nc.sync.dma_start(out_d, sim_sb[:])

    nc.compile()
    return nc


_NC = None


def _get_nc():
    global _NC
    if _NC is None:
        _NC = build_kernel()
    return _NC


def make_in_maps(inputs):
    import ml_dtypes

    bf16 = ml_dtypes.bfloat16
    he_q = np.asarray(inputs["he_ques"]).astype(np.int32)   # [64, 32]
    he_k = np.asarray(inputs["he_kg"]).astype(np.int32)     # [64, 256]
    emb0 = np.asarray(inputs["emb"], dtype=np.float32)
    emb = np.zeros((VOCAB, EA), dtype=bf16)
    emb[:, :E] = emb0.astype(bf16)
    emb[:, E] = 1.0                                         # ones col at 300
    wq = np.zeros((EA, H), dtype=bf16)
    wq[:E] = np.asarray(inputs["Wq"], np.float32).astype(bf16)
    wq[E] = np.asarray(inputs["bq"], np.float32).astype(bf16)
    wk = np.zeros((EA, H), dtype=bf16)
    wk[:E] = np.asarray(inputs["Wk"], np.float32).astype(bf16)
    wk[E] = np.asarray(inputs["bk"], np.float32).astype(bf16)
    watt = np.ascontiguousarray(
        np.asarray(inputs["Watt"], np.float32).reshape(DT, 128, G)
        .transpose(1, 0, 2)).astype(bf16)                   # [128, DT, G]
    wout = np.ascontiguousarray(
        np.asarray(inputs["Wout"], np.float32)).astype(bf16)
    bout = np.ascontiguousarray(
        np.broadcast_to(np.asarray(inputs["bout"], np.float32), (BL, N_OUT)))
    glovet0 = np.asarray(inputs["glove_cands"], np.float32).T  # [300, 4000]
    glovet = np.zeros((E_CH, NA_CH, 128, NA_W), dtype=bf16)
    for c in range(E_CH):
        rows = min(128, N_OUT - c * 128)
        for a in range(NA_CH):
            glovet[c, a, :rows, :] = glovet0[
                c * 128 : c * 128 + rows, a * NA_W : (a + 1) * NA_W
            ].astype(bf16)
    ident = np.eye(128, dtype=bf16)

    in_maps = []
    for i in range(N_CORES):
        iq = he_q[i * BL : (i + 1) * BL].reshape(-1)        # [256]
        ik = he_k[i * BL : (i + 1) * BL].reshape(-1)        # [2048]
        in_maps.append({
            "emb": emb,
            "idx_q": np.ascontiguousarray(iq.reshape(TQ_TILES, 128).T),
            "idx_k": np.ascontiguousarray(ik.reshape(TK_TILES, 128).T),
            "wq": wq,
            "wk": wk,
            "watt": watt,
            "wout": wout,
            "bout": bout,
            "glovet": glovet,
            "ident": ident,
        })
    return in_maps


def kernel(**inputs) -> np.ndarray:
    nc = _get_nc()
    in_maps = make_in_maps(inputs)
    res = run_bass_kernel_spmd(nc, in_maps, list(range(N_CORES)))
    return np.concatenate([res.results[i]["out"] for i in range(N_CORES)], axis=0)
